# revision 13
# baseline (speedup 1.0000x reference)
"""Trainium2 Bass kernel for patch-SSD keypoint matching.

Reference computation (see problem): for each of 512 keypoints, compare a
4x4x4 (stride-2) patch of feat_fix around the keypoint against the same
patch of feat_mov displaced by each of 9x9x9 (stride-2) displacements.
Output cost[N, 1, 9, 9, 9] = mean squared difference over (C=12, P=64),
plus the constant displacement grid [729, 3].

Strategy (8 NeuronCores, data-parallel over keypoints, 64 kps/core):
  * Union of (patch offset + displacement) positions per keypoint is a
    12^3 stride-2 grid -> gather only C*12^3 values per keypoint.
  * cost = (sum pf^2 + box(sum_c pm^2) - 2*corr(pf, pm)) / 768.  The whole
    (term2 - 2*corr) is computed by one block-diagonal matmul pipeline:
    z-handling fused into 4 PSUM-accumulated shifted-rhs matmuls, y/x folds
    by constant 0/1 selector matmuls.
  * Gather: per-keypoint dynamic-offset DMA (base address from registers
    loaded from an SBUF offsets tensor -> SPMD-safe) out of a host-packed
    volume: y/x parity-packed + overlapping x-strips so that each (c, z)
    needs exactly one contiguous 320-element span.
"""

import sys
import threading

if "/opt/trn_rl_repo" not in sys.path:
    sys.path.insert(0, "/opt/trn_rl_repo")

import numpy as np

import concourse.bass as bass
import concourse.bacc as bacc
import concourse.tile as tile
from concourse import mybir
from concourse.bass_utils import run_bass_kernel_spmd

# problem constants
C, D, H, W = 12, 128, 160, 160
N_KPS = 512
DISP_RADIUS, DISP_STEP = 8, 2
PATCH_RADIUS, PATCH_STEP = 3, 2
LW = 9          # displacements per dim
NP_ = 4         # patch taps per dim
G12 = 12        # combined grid points per dim (LW + NP_ - 1)
L = LW ** 3     # 729
P64 = NP_ ** 3  # 64

N_CORES = 8
KP_PER_CORE = N_KPS // N_CORES  # 64
BK = 6                           # keypoints per compute batch
NB = (KP_PER_CORE + BK - 1) // BK  # 11 batches (66 slots, last 2 padded)

# packed-volume geometry
HP = H // 2            # 80 (parity-packed y')
WP = W // 2            # 80 (parity-packed x')
NS = 4                 # x-strips
SW = 28                # strip width
SSTRIDE = 16           # strip stride
SPAN = (G12 - 1) * SW + G12          # 320 contiguous elems per (c, z)
MOVF = 4 * NS * HP * SW              # 35840 flat per (c, z): (par, s, y', x'')
FIXF = 4 * HP * WP                   # 25600 flat per (c, z): (par, y', x')
FSPAN = (NP_ - 1) * WP + NP_         # 244 contiguous per (c, z) for fix

_build_lock = threading.Lock()
_cache = {}


def _pack_mov(feat_mov):
    """[1,C,D,H,W] f32 -> [C, D, MOVF] with (par, strip, y', x'') packing."""
    fm = np.ascontiguousarray(feat_mov[0])             # [C, D, H, W]
    # split parities: [C, D, HP, yp, WP, xp] -> par = yp*2+xp
    t = fm.reshape(C, D, HP, 2, WP, 2)
    t = np.transpose(t, (0, 1, 3, 5, 2, 4))             # [C, D, 2, 2, HP, WP]
    t = t.reshape(C, D, 4, HP, WP)
    v = np.zeros((C, D, 4, NS, HP, SW), dtype=np.float32)
    for s in range(NS):
        w = min(SW, WP - SSTRIDE * s)
        v[:, :, :, s, :, :w] = t[:, :, :, :, SSTRIDE * s:SSTRIDE * s + w]
    return np.ascontiguousarray(v.reshape(C, D, MOVF))


def _pack_fix(feat_fix):
    """[1,C,D,H,W] f32 -> [C, D, FIXF] with (par, y', x') packing."""
    ff = np.ascontiguousarray(feat_fix[0])
    t = ff.reshape(C, D, HP, 2, WP, 2)
    t = np.transpose(t, (0, 1, 3, 5, 2, 4)).reshape(C, D, 4, HP, WP)
    return np.ascontiguousarray(t.reshape(C, D, FIXF))


def _offsets_for(kps):
    """kps [n,3] int -> (mov_z0, mov_rem, fix_z0, fix_rem) int32 arrays."""
    kz = kps[:, 0].astype(np.int64)
    ky = kps[:, 1].astype(np.int64)
    kx = kps[:, 2].astype(np.int64)
    z0 = kz - (DISP_RADIUS + PATCH_RADIUS)
    y0 = ky - (DISP_RADIUS + PATCH_RADIUS)
    x0 = kx - (DISP_RADIUS + PATCH_RADIUS)
    par = (y0 & 1) * 2 + (x0 & 1)
    yq = y0 >> 1
    xq = x0 >> 1
    s = xq >> 4
    xr = xq & 15
    mov_z0 = z0
    mov_rem = par * (NS * HP * SW) + s * (HP * SW) + yq * SW + xr
    fix_z0 = kz - PATCH_RADIUS
    fix_rem = par * (HP * WP) + (yq + 4) * WP + (xq + 4)
    return (mov_z0.astype(np.int32), mov_rem.astype(np.int32),
            fix_z0.astype(np.int32), fix_rem.astype(np.int32))


def _selectors():
    """Constant block-diagonal / selector matrices (host-built)."""
    ones_bd = np.zeros((BK * C, BK), dtype=np.float32)
    for k in range(BK):
        ones_bd[C * k:C * k + C, k] = 1.0
    # y-fold input rows: [0:96] = cost3 rows, [96:102] = box rows
    sel_y = np.zeros((BK * 16 + BK, 4, BK * 4 + BK), dtype=np.float32)
    for k in range(BK):
        for pyv in range(4):
            for px in range(4):
                sel_y[16 * k + 4 * pyv + px, pyv, 4 * k + px] = 1.0
        for pyv in range(4):
            sel_y[BK * 16 + k, pyv, BK * 4 + k] = 1.0
    sel_x = np.zeros((BK * 4 + BK, 4, BK), dtype=np.float32)
    for k in range(BK):
        for px in range(4):
            sel_x[4 * k + px, px, k] = 1.0
        for px in range(4):
            sel_x[BK * 4 + k, px, k] = 1.0
    # block-diagonal -2 mask for the lhsT build: row (kp,c), col (kp',py,px)
    pfmask = np.zeros((BK * C, BK * 16), dtype=np.float32)
    for k in range(BK):
        pfmask[C * k:C * k + C, 16 * k:16 * k + 16] = -2.0
    return (ones_bd,
            np.ascontiguousarray(sel_y.reshape(BK * 16 + BK, 4 * (BK * 4 + BK))),
            np.ascontiguousarray(sel_x.reshape(BK * 4 + BK, 4 * BK)),
            pfmask)


def _build_nc():
    """Build + compile the SPMD kernel once (shapes are static)."""
    fp32 = mybir.dt.float32
    i32 = mybir.dt.int32
    KC = BK * C                      # 72 partitions (kp, c)
    M1 = BK * 16                     # 96 z-stage output partitions
    MY = BK * 4 + BK                 # 30 y-fold output partitions
    NYR = BK * 16 + BK               # 102 y-fold input partitions (cost3@[0:96], box@[96:102])

    nc = bacc.Bacc("TRN2", target_bir_lowering=False, debug=False)
    vm = nc.dram_tensor("vmov", [C, D, MOVF], fp32, kind="ExternalInput")
    vf = nc.dram_tensor("vfix", [C, D, FIXF], fp32, kind="ExternalInput")
    offs = nc.dram_tensor("offs", [4, NB * BK], i32, kind="ExternalInput")
    ones_d = nc.dram_tensor("ones_bd", [KC, BK], fp32, kind="ExternalInput")
    sely_d = nc.dram_tensor("sel_y", [NYR, 4 * MY], fp32, kind="ExternalInput")
    selx_d = nc.dram_tensor("sel_x", [MY, 4 * BK], fp32, kind="ExternalInput")
    pfmask_d = nc.dram_tensor("pfmask", [KC, M1], fp32, kind="ExternalInput")
    out_d = nc.dram_tensor("cost", [NB * BK, L], fp32, kind="ExternalOutput")

    with tile.TileContext(nc) as tc:
        with (
            tc.tile_pool(name="const", bufs=1) as cpool,
            tc.tile_pool(name="gather", bufs=2) as gpool,
            tc.tile_pool(name="work", bufs=2) as wpool,
            tc.tile_pool(name="lhs", bufs=1) as lpool,
            tc.tile_pool(name="psB", bufs=1, space="PSUM") as psB,
            tc.tile_pool(name="psC", bufs=1, space="PSUM") as psC,
            tc.tile_pool(name="psD", bufs=1, space="PSUM") as psD,
        ):
            offs_sb = cpool.tile([4, NB * BK], i32)
            nc.sync.dma_start(offs_sb[:], offs[:])
            ones_sb = cpool.tile([KC, BK], fp32)
            nc.sync.dma_start(ones_sb[:], ones_d[:])
            sely_sb = cpool.tile([NYR, 4 * MY], fp32)
            nc.sync.dma_start(sely_sb[:], sely_d[:])
            selx_sb = cpool.tile([MY, 4 * BK], fp32)
            nc.sync.dma_start(selx_sb[:], selx_d[:])
            pfmask_sb = cpool.tile([KC, M1], fp32)
            nc.sync.dma_start(pfmask_sb[:], pfmask_d[:])

            lhsT = lpool.tile([KC, 4, M1], fp32)       # -2*PF block-diag, per pz

            for b in range(NB):
                # ---- gathers -------------------------------------------------
                g6 = gpool.tile([KC, G12, SPAN], fp32, tag="g6")
                pf6 = gpool.tile([KC, NP_, FSPAN], fp32, tag="pf6")
                for k in range(BK):
                    j = b * BK + k
                    with (nc.sync.register(f"mz{b}_{k}") as rz,
                          nc.sync.register(f"mr{b}_{k}") as rr):
                        nc.sync.reg_load(rz, offs_sb[0:1, j:j + 1])
                        nc.sync.reg_load(rr, offs_sb[1:2, j:j + 1])
                        zv = nc.sync.snap(rz, min_val=0, max_val=D - 2 * G12 + 1)
                        rv = nc.sync.snap(rr, min_val=0, max_val=MOVF - SPAN)
                        nc.sync.dma_start(
                            g6[C * k:C * k + C, :, :],
                            vm[:, bass.ds(zv, G12, 2), bass.ds(rv, SPAN)])
                    with (nc.scalar.register(f"fz{b}_{k}") as rz2,
                          nc.scalar.register(f"fr{b}_{k}") as rr2):
                        nc.scalar.reg_load(rz2, offs_sb[2:3, j:j + 1])
                        nc.scalar.reg_load(rr2, offs_sb[3:4, j:j + 1])
                        zv2 = nc.scalar.snap(rz2, min_val=0, max_val=D - 2 * NP_ + 1)
                        rv2 = nc.scalar.snap(rr2, min_val=0, max_val=FIXF - FSPAN)
                        nc.scalar.dma_start(
                            pf6[C * k:C * k + C, :, :],
                            vf[:, bass.ds(zv2, NP_, 2), bass.ds(rv2, FSPAN)])

                # ---- lhsT build: -2 * PF block-diag via broadcast * mask
                for pz in range(4):
                    src = bass.AP(pf6.tensor, pz * FSPAN,
                                  [[pf6[:].ap[0][0], KC],
                                   [0, BK], [WP, NP_], [1, NP_]])
                    nc.vector.tensor_tensor(
                        out=lhsT[:, pz, :], in0=src, in1=pfmask_sb[:],
                        op=mybir.AluOpType.mult)

                # ---- squared-moving path ------------------------------------
                g2 = wpool.tile([KC, G12 * G12 * G12], fp32, tag="g2")
                src = bass.AP(g6.tensor, g6[0, 0, 0].offset,
                              [[g6[:].ap[0][0], KC],
                               [SPAN, G12], [SW, G12], [1, G12]])
                nc.scalar.activation(g2[:], src, mybir.ActivationFunctionType.Square)
                zb1 = wpool.tile([KC, LW * G12 * G12], fp32, tag="zb1")
                zb2 = wpool.tile([KC, LW * G12 * G12], fp32, tag="zb2")
                ZR = G12 * G12  # 144
                nc.vector.tensor_add(
                    zb1[:], g2[:, 0:LW * ZR], g2[:, ZR:(LW + 1) * ZR])
                nc.vector.tensor_add(
                    zb2[:], g2[:, 2 * ZR:(LW + 2) * ZR], g2[:, 3 * ZR:(LW + 3) * ZR])
                nc.vector.tensor_add(zb1[:], zb1[:], zb2[:])

                # ---- z-stage + box c-reduction into PSUM B -------------------
                Bp = psB.tile([M1, 3 * 512], fp32, tag="B")
                for pz in range(4):
                    for ck in range(3):
                        rhs = bass.AP(g6.tensor, g6[0, pz + 3 * ck, 0].offset,
                                      [[g6[:].ap[0][0], KC],
                                       [SPAN, 3], [SW, G12], [1, G12]])
                        nc.tensor.matmul(
                            Bp[0:M1, 512 * ck:512 * ck + 432],
                            lhsT[:, pz, :], rhs,
                            start=(pz == 0), stop=(pz == 3))
                boxP = psC.tile([MY, 3 * 512], fp32, tag="C")
                for ck in range(3):
                    nc.tensor.matmul(
                        boxP[0:BK, 512 * ck:512 * ck + 432],
                        ones_sb[:], zb1[:, 432 * ck:432 * ck + 432],
                        start=True, stop=True)

                # ---- B -> SBUF (cost3 rows via ACT, box rows via DVE quadrant move)
                bsb = wpool.tile([NYR, 1296], fp32, tag="bsb")
                nc.scalar.copy(
                    bsb[0:M1, :],
                    bass.AP(Bp.tensor, Bp[0, 0].offset,
                            [[Bp[:].ap[0][0], M1], [512, 3], [1, 432]]))
                nc.vector.tensor_copy(
                    bsb[M1:M1 + BK, :],
                    bass.AP(boxP.tensor, boxP[0, 0].offset,
                            [[boxP[:].ap[0][0], BK], [512, 3], [1, 432]]))

                # ---- y-fold --------------------------------------------------
                Cp = psC.tile([MY, 3 * 512], fp32, tag="C")
                ycw = (432, 432, 108)
                ycl = (4, 4, 1)
                for pyv in range(4):
                    lz0 = 0
                    for ck in range(3):
                        rhs = bass.AP(bsb.tensor, bsb[0, lz0 * ZR + pyv * G12].offset,
                                      [[bsb[:].ap[0][0], NYR],
                                       [ZR, ycl[ck]], [G12, LW], [1, G12]])
                        nc.tensor.matmul(
                            Cp[:, 512 * ck:512 * ck + ycw[ck]],
                            sely_sb[:, MY * pyv:MY * pyv + MY], rhs,
                            start=(pyv == 0), stop=(pyv == 3))
                        lz0 += ycl[ck]
                csb = wpool.tile([MY, LW * LW * G12], fp32, tag="csb")
                nc.scalar.copy(
                    csb[:, 0:864],
                    bass.AP(Cp.tensor, Cp[0, 0].offset,
                            [[Cp[:].ap[0][0], MY], [512, 2], [1, 432]]))
                nc.scalar.copy(csb[:, 864:972], Cp[:, 1024:1132])

                # ---- x-fold + term1 -----------------------------------------
                Dp = psD.tile([BK, 1024], fp32, tag="D")
                xcw = (486, 243)
                xcl = (6, 3)
                YR = LW * G12  # 108
                for px in range(4):
                    lz0 = 0
                    for ck in range(2):
                        rhs = bass.AP(csb.tensor, csb[0, lz0 * YR + px].offset,
                                      [[csb[:].ap[0][0], MY],
                                       [YR, xcl[ck]], [G12, LW], [1, LW]])
                        nc.tensor.matmul(
                            Dp[:, 512 * ck:512 * ck + xcw[ck]],
                            selx_sb[:, BK * px:BK * px + BK], rhs,
                            start=(px == 0), stop=(px == 3))
                        lz0 += xcl[ck]

                pfsq = wpool.tile([KC, P64], fp32, tag="pfsq")
                src = bass.AP(pf6.tensor, pf6[0, 0, 0].offset,
                              [[pf6[:].ap[0][0], KC],
                               [FSPAN, NP_], [WP, NP_], [1, NP_]])
                nc.scalar.activation(pfsq[:], src, mybir.ActivationFunctionType.Square)
                nc.tensor.matmul(Dp[:, 768:768 + P64], ones_sb[:], pfsq[:],
                                 start=True, stop=True)
                t1 = wpool.tile([BK, 1], fp32, tag="t1")
                nc.vector.tensor_reduce(t1[:], Dp[:, 768:768 + P64],
                                        axis=mybir.AxisListType.XYZW,
                                        op=mybir.AluOpType.add)
                t1s = wpool.tile([BK, 1], fp32, tag="t1s")
                nc.vector.tensor_scalar_mul(t1s[:], t1[:], 1.0 / (C * P64))

                # ---- combine + store ----------------------------------------
                cost_sb = wpool.tile([BK, L], fp32, tag="cost")
                nc.scalar.activation(cost_sb[:, 0:486], Dp[:, 0:486],
                                     mybir.ActivationFunctionType.Identity,
                                     bias=t1s[:], scale=1.0 / (C * P64))
                nc.scalar.activation(cost_sb[:, 486:729], Dp[:, 512:755],
                                     mybir.ActivationFunctionType.Identity,
                                     bias=t1s[:], scale=1.0 / (C * P64))
                nc.sync.dma_start(out_d[b * BK:(b + 1) * BK, :], cost_sb[:])

    nc.compile()
    return nc


def _disp():
    a = np.arange(-DISP_RADIUS, DISP_RADIUS + 1, DISP_STEP)
    g = np.stack(np.meshgrid(a, a, a, indexing="ij"), axis=-1)
    return g.reshape(-1, 3).astype(np.float32)


def kernel(fix_kps, feat_fix, feat_mov):
    fix_kps = np.asarray(fix_kps)
    feat_fix = np.asarray(feat_fix, dtype=np.float32)
    feat_mov = np.asarray(feat_mov, dtype=np.float32)

    with _build_lock:
        if "nc" not in _cache:
            _cache["nc"] = _build_nc()
    nc = _cache["nc"]

    vmov = _pack_mov(feat_mov)
    vfix = _pack_fix(feat_fix)
    ones_bd, sel_y, sel_x, pfmask = _selectors()

    in_maps = []
    for c in range(N_CORES):
        kps = fix_kps[c * KP_PER_CORE:(c + 1) * KP_PER_CORE]
        pad = np.repeat(kps[-1:], NB * BK - KP_PER_CORE, axis=0)
        kpad = np.concatenate([kps, pad], axis=0)
        mz, mr, fz, fr = _offsets_for(kpad)
        offs = np.stack([mz, mr, fz, fr]).astype(np.int32)
        in_maps.append({
            "vmov": vmov, "vfix": vfix, "offs": offs,
            "ones_bd": ones_bd, "sel_y": sel_y, "sel_x": sel_x,
            "pfmask": pfmask,
        })

    res = run_bass_kernel_spmd(nc, in_maps, list(range(N_CORES)))
    cost = np.concatenate(
        [res.results[c]["cost"][:KP_PER_CORE] for c in range(N_CORES)], axis=0)
    return cost.reshape(N_KPS, 1, LW, LW, LW), _disp()


# revision 16
# speedup vs baseline: 2.3499x; 2.3499x over previous
"""Trainium2 Bass kernel for patch-SSD keypoint matching.

Reference computation (see problem): for each of 512 keypoints, compare a
4x4x4 (stride-2) patch of feat_fix around the keypoint against the same
patch of feat_mov displaced by each of 9x9x9 (stride-2) displacements.
Output cost[N, 1, 9, 9, 9] = mean squared difference over (C=12, P=64),
plus the constant displacement grid [729, 3].

Strategy (8 NeuronCores, data-parallel over keypoints, 64 kps/core):
  * Union of (patch offset + displacement) positions per keypoint is a
    12^3 stride-2 grid -> gather only C*12^3 values per keypoint.
  * cost = (sum pf^2 + box(sum_c pm^2) - 2*corr(pf, pm)) / 768.  The whole
    (term2 - 2*corr) is computed by one block-diagonal matmul pipeline:
    z-handling fused into 4 PSUM-accumulated shifted-rhs matmuls, y/x folds
    by constant 0/1 selector matmuls.
  * Gather: per-keypoint dynamic-offset DMA (base address from registers
    loaded from an SBUF offsets tensor -> SPMD-safe) out of a host-packed
    volume: y/x parity-packed + overlapping x-strips so that each (c, z)
    needs exactly one contiguous 320-element span.
"""

import sys
import threading

if "/opt/trn_rl_repo" not in sys.path:
    sys.path.insert(0, "/opt/trn_rl_repo")

import ml_dtypes
import numpy as np

import concourse.bass as bass
import concourse.bacc as bacc
import concourse.tile as tile
from concourse import mybir
from concourse.bass_utils import run_bass_kernel_spmd

# problem constants
C, D, H, W = 12, 128, 160, 160
N_KPS = 512
DISP_RADIUS, DISP_STEP = 8, 2
PATCH_RADIUS, PATCH_STEP = 3, 2
LW = 9          # displacements per dim
NP_ = 4         # patch taps per dim
G12 = 12        # combined grid points per dim (LW + NP_ - 1)
L = LW ** 3     # 729
P64 = NP_ ** 3  # 64

N_CORES = 8
KP_PER_CORE = N_KPS // N_CORES  # 64
BK = 6                           # keypoints per compute batch
NB = (KP_PER_CORE + BK - 1) // BK  # 11 batches (66 slots, last 2 padded)

# packed-volume geometry
HP = H // 2            # 80 (parity-packed y')
WP = W // 2            # 80 (parity-packed x')
NS = 4                 # x-strips
SW = 28                # strip width
SSTRIDE = 16           # strip stride
SPAN = (G12 - 1) * SW + G12          # 320 contiguous elems per (c, z)
MOVF = 4 * NS * HP * SW              # 35840 flat per (c, z): (par, s, y', x'')
FIXF = 4 * HP * WP                   # 25600 flat per (c, z): (par, y', x')
FSPAN = (NP_ - 1) * WP + NP_         # 244 contiguous per (c, z) for fix

_build_lock = threading.Lock()
_cache = {}


def _pack_mov(feat_mov):
    """[1,C,D,H,W] f32 -> bf16 [C, D, MOVF] with (par, strip, y', x'') packing."""
    fm = np.ascontiguousarray(feat_mov[0]).astype(ml_dtypes.bfloat16)
    t = fm.reshape(C, D, HP, 2, WP, 2)
    t = np.transpose(t, (0, 1, 3, 5, 2, 4))             # [C, D, 2, 2, HP, WP]
    t = t.reshape(C, D, 4, HP, WP)
    v = np.zeros((C, D, 4, NS, HP, SW), dtype=ml_dtypes.bfloat16)
    for s in range(NS):
        w = min(SW, WP - SSTRIDE * s)
        v[:, :, :, s, :, :w] = t[:, :, :, :, SSTRIDE * s:SSTRIDE * s + w]
    return np.ascontiguousarray(v.reshape(C, D, MOVF))


def _pack_fix(feat_fix):
    """[1,C,D,H,W] f32 -> bf16 [C, D, FIXF] with (par, y', x') packing."""
    ff = np.ascontiguousarray(feat_fix[0]).astype(ml_dtypes.bfloat16)
    t = ff.reshape(C, D, HP, 2, WP, 2)
    t = np.transpose(t, (0, 1, 3, 5, 2, 4)).reshape(C, D, 4, HP, WP)
    return np.ascontiguousarray(t.reshape(C, D, FIXF))


def _offsets_for(kps):
    """kps [n,3] int -> (mov_z0, mov_rem, fix_z0, fix_rem) int32 arrays."""
    kz = kps[:, 0].astype(np.int64)
    ky = kps[:, 1].astype(np.int64)
    kx = kps[:, 2].astype(np.int64)
    z0 = kz - (DISP_RADIUS + PATCH_RADIUS)
    y0 = ky - (DISP_RADIUS + PATCH_RADIUS)
    x0 = kx - (DISP_RADIUS + PATCH_RADIUS)
    par = (y0 & 1) * 2 + (x0 & 1)
    yq = y0 >> 1
    xq = x0 >> 1
    s = xq >> 4
    xr = xq & 15
    mov_z0 = z0
    mov_rem = par * (NS * HP * SW) + s * (HP * SW) + yq * SW + xr
    fix_z0 = kz - PATCH_RADIUS
    fix_rem = par * (HP * WP) + (yq + 4) * WP + (xq + 4)
    return (mov_z0.astype(np.int32), mov_rem.astype(np.int32),
            fix_z0.astype(np.int32), fix_rem.astype(np.int32))


def _selectors():
    """Constant block-diagonal / selector matrices (host-built)."""
    ones_bd = np.zeros((BK * C, BK), dtype=np.float32)
    for k in range(BK):
        ones_bd[C * k:C * k + C, k] = 1.0
    # y-fold input rows: [0:96] = cost3 rows, [96:102] = box rows
    sel_y = np.zeros((BK * 16 + BK, 4, BK * 4 + BK), dtype=np.float32)
    for k in range(BK):
        for pyv in range(4):
            for px in range(4):
                sel_y[16 * k + 4 * pyv + px, pyv, 4 * k + px] = 1.0
        for pyv in range(4):
            sel_y[BK * 16 + k, pyv, BK * 4 + k] = 1.0
    sel_x = np.zeros((BK * 4 + BK, 4, BK), dtype=np.float32)
    for k in range(BK):
        for px in range(4):
            sel_x[4 * k + px, px, k] = 1.0
        for px in range(4):
            sel_x[BK * 4 + k, px, k] = 1.0
    # block-diagonal -2 mask for the lhsT build: row (kp,c), col (kp',py,px)
    pfmask = np.zeros((BK * C, BK * 16), dtype=np.float32)
    for k in range(BK):
        pfmask[C * k:C * k + C, 16 * k:16 * k + 16] = -2.0
    b16 = ml_dtypes.bfloat16
    return (ones_bd.astype(b16),
            np.ascontiguousarray(sel_y.reshape(BK * 16 + BK, 4 * (BK * 4 + BK))).astype(b16),
            np.ascontiguousarray(sel_x.reshape(BK * 4 + BK, 4 * BK)).astype(b16),
            pfmask.astype(b16))


def _build_nc():
    """Build + compile the SPMD kernel once (shapes are static)."""
    fp32 = mybir.dt.float32
    bf16 = mybir.dt.bfloat16
    i32 = mybir.dt.int32
    KC = BK * C                      # 72 partitions (kp, c)
    M1 = BK * 16                     # 96 z-stage output partitions
    MY = BK * 4 + BK                 # 30 y-fold output partitions
    NYR = BK * 16 + BK               # 102 y-fold input partitions (cost3@[0:96], box@[96:102])

    nc = bacc.Bacc("TRN2", target_bir_lowering=False, debug=False)
    vm = nc.dram_tensor("vmov", [C, D, MOVF], bf16, kind="ExternalInput")
    vf = nc.dram_tensor("vfix", [C, D, FIXF], bf16, kind="ExternalInput")
    offs = nc.dram_tensor("offs", [2, 2 * NB * BK], i32, kind="ExternalInput")
    ones_d = nc.dram_tensor("ones_bd", [KC, BK], bf16, kind="ExternalInput")
    sely_d = nc.dram_tensor("sel_y", [NYR, 4 * MY], bf16, kind="ExternalInput")
    selx_d = nc.dram_tensor("sel_x", [MY, 4 * BK], bf16, kind="ExternalInput")
    pfmask_d = nc.dram_tensor("pfmask", [KC, M1], bf16, kind="ExternalInput")
    out_d = nc.dram_tensor("cost", [NB * BK, L], fp32, kind="ExternalOutput")

    with tile.TileContext(nc) as tc:
        with (
            tc.tile_pool(name="const", bufs=1) as cpool,
            tc.tile_pool(name="gather", bufs=3) as gpool,
            tc.tile_pool(name="work", bufs=3) as wpool,
            tc.tile_pool(name="lhs", bufs=2) as lpool,
            tc.tile_pool(name="psB", bufs=1, space="PSUM") as psB,
            tc.tile_pool(name="psC", bufs=1, space="PSUM") as psC,
            tc.tile_pool(name="psD", bufs=1, space="PSUM") as psD,
        ):
            offs_sb = cpool.tile([2, 2 * NB * BK], i32)
            nc.sync.dma_start(offs_sb[:], offs[:])
            ones_sb = cpool.tile([KC, BK], bf16)
            nc.sync.dma_start(ones_sb[:], ones_d[:])
            sely_sb = cpool.tile([NYR, 4 * MY], bf16)
            nc.sync.dma_start(sely_sb[:], sely_d[:])
            selx_sb = cpool.tile([MY, 4 * BK], bf16)
            nc.sync.dma_start(selx_sb[:], selx_d[:])
            pfmask_sb = cpool.tile([KC, M1], bf16)
            nc.sync.dma_start(pfmask_sb[:], pfmask_d[:])

            for b in range(NB):
                # ---- gathers -------------------------------------------------
                g6 = gpool.tile([KC, G12, SPAN], bf16, tag="g6")
                pf6 = gpool.tile([KC, NP_, FSPAN], bf16, tag="pf6")
                for k in range(BK):
                    j = b * BK + k
                    eng = nc.sync if (k % 2 == 0) else nc.scalar
                    with (eng.register(f"mz{b}_{k}") as rz,
                          eng.register(f"mr{b}_{k}") as rr):
                        eng.reg_load([rz, rr], offs_sb[0:1, 2 * j:2 * j + 2])
                        zv = eng.snap(rz, min_val=0, max_val=D - 2 * G12 + 1)
                        rv = eng.snap(rr, min_val=0, max_val=MOVF - SPAN)
                        eng.dma_start(
                            g6[C * k:C * k + C, :, :],
                            vm[:, bass.ds(zv, G12, 2), bass.ds(rv, SPAN)])
                    eng2 = nc.scalar if (k % 2 == 0) else nc.sync
                    with (eng2.register(f"fz{b}_{k}") as rz2,
                          eng2.register(f"fr{b}_{k}") as rr2):
                        eng2.reg_load([rz2, rr2], offs_sb[1:2, 2 * j:2 * j + 2])
                        zv2 = eng2.snap(rz2, min_val=0, max_val=D - 2 * NP_ + 1)
                        rv2 = eng2.snap(rr2, min_val=0, max_val=FIXF - FSPAN)
                        eng2.dma_start(
                            pf6[C * k:C * k + C, :, :],
                            vf[:, bass.ds(zv2, NP_, 2), bass.ds(rv2, FSPAN)])

                # ---- lhsT build: -2 * PF block-diag via broadcast * mask
                lhsT = lpool.tile([KC, 4, M1], bf16, tag="lhsT")
                for pz in range(4):
                    src = bass.AP(pf6.tensor, pz * FSPAN,
                                  [[pf6[:].ap[0][0], KC],
                                   [0, BK], [WP, NP_], [1, NP_]])
                    nc.vector.tensor_tensor(
                        out=lhsT[:, pz, :], in0=src, in1=pfmask_sb[:],
                        op=mybir.AluOpType.mult)

                # ---- squared-moving path ------------------------------------
                g2 = wpool.tile([KC, G12 * G12 * G12], bf16, tag="g2")
                src = bass.AP(g6.tensor, g6[0, 0, 0].offset,
                              [[g6[:].ap[0][0], KC],
                               [SPAN, G12], [SW, G12], [1, G12]])
                nc.scalar.activation(g2[:], src, mybir.ActivationFunctionType.Square)
                zb1 = wpool.tile([KC, LW * G12 * G12], bf16, tag="zb1")
                zb2 = wpool.tile([KC, LW * G12 * G12], bf16, tag="zb2")
                ZR = G12 * G12  # 144
                nc.vector.tensor_add(
                    zb1[:], g2[:, 0:LW * ZR], g2[:, ZR:(LW + 1) * ZR])
                nc.vector.tensor_add(
                    zb2[:], g2[:, 2 * ZR:(LW + 2) * ZR], g2[:, 3 * ZR:(LW + 3) * ZR])
                nc.vector.tensor_add(zb1[:], zb1[:], zb2[:])

                # ---- z-stage + box c-reduction into PSUM B -------------------
                Bp = psB.tile([M1, 3 * 512], fp32, tag="B")
                for pz in range(4):
                    for ck in range(3):
                        rhs = bass.AP(g6.tensor, g6[0, pz + 3 * ck, 0].offset,
                                      [[g6[:].ap[0][0], KC],
                                       [SPAN, 3], [SW, G12], [1, G12]])
                        nc.tensor.matmul(
                            Bp[0:M1, 512 * ck:512 * ck + 432],
                            lhsT[:, pz, :], rhs,
                            start=(pz == 0), stop=(pz == 3))
                boxP = psC.tile([MY, 3 * 512], fp32, tag="C")
                for ck in range(3):
                    nc.tensor.matmul(
                        boxP[0:BK, 512 * ck:512 * ck + 432],
                        ones_sb[:], zb1[:, 432 * ck:432 * ck + 432],
                        start=True, stop=True)

                # ---- B -> SBUF (cost3 rows via ACT, box rows via DVE quadrant move)
                bsb = wpool.tile([NYR, 1296], bf16, tag="bsb")
                nc.scalar.copy(
                    bsb[0:M1, :],
                    bass.AP(Bp.tensor, Bp[0, 0].offset,
                            [[Bp[:].ap[0][0], M1], [512, 3], [1, 432]]))
                nc.vector.tensor_copy(
                    bsb[M1:M1 + BK, :],
                    bass.AP(boxP.tensor, boxP[0, 0].offset,
                            [[boxP[:].ap[0][0], BK], [512, 3], [1, 432]]))

                # ---- y-fold --------------------------------------------------
                Cp = psC.tile([MY, 3 * 512], fp32, tag="C")
                ycw = (432, 432, 108)
                ycl = (4, 4, 1)
                for pyv in range(4):
                    lz0 = 0
                    for ck in range(3):
                        rhs = bass.AP(bsb.tensor, bsb[0, lz0 * ZR + pyv * G12].offset,
                                      [[bsb[:].ap[0][0], NYR],
                                       [ZR, ycl[ck]], [G12, LW], [1, G12]])
                        nc.tensor.matmul(
                            Cp[:, 512 * ck:512 * ck + ycw[ck]],
                            sely_sb[:, MY * pyv:MY * pyv + MY], rhs,
                            start=(pyv == 0), stop=(pyv == 3))
                        lz0 += ycl[ck]
                csb = wpool.tile([MY, LW * LW * G12], bf16, tag="csb")
                nc.scalar.copy(
                    csb[:, 0:864],
                    bass.AP(Cp.tensor, Cp[0, 0].offset,
                            [[Cp[:].ap[0][0], MY], [512, 2], [1, 432]]))
                nc.scalar.copy(csb[:, 864:972], Cp[:, 1024:1132])

                # ---- x-fold + term1 -----------------------------------------
                Dp = psD.tile([BK, 1024], fp32, tag="D")
                xcw = (486, 243)
                xcl = (6, 3)
                YR = LW * G12  # 108
                for px in range(4):
                    lz0 = 0
                    for ck in range(2):
                        rhs = bass.AP(csb.tensor, csb[0, lz0 * YR + px].offset,
                                      [[csb[:].ap[0][0], MY],
                                       [YR, xcl[ck]], [G12, LW], [1, LW]])
                        nc.tensor.matmul(
                            Dp[:, 512 * ck:512 * ck + xcw[ck]],
                            selx_sb[:, BK * px:BK * px + BK], rhs,
                            start=(px == 0), stop=(px == 3))
                        lz0 += xcl[ck]

                pfsq = wpool.tile([KC, P64], bf16, tag="pfsq")
                src = bass.AP(pf6.tensor, pf6[0, 0, 0].offset,
                              [[pf6[:].ap[0][0], KC],
                               [FSPAN, NP_], [WP, NP_], [1, NP_]])
                nc.scalar.activation(pfsq[:], src, mybir.ActivationFunctionType.Square)
                nc.tensor.matmul(Dp[:, 768:768 + P64], ones_sb[:], pfsq[:],
                                 start=True, stop=True)
                t1 = wpool.tile([BK, 1], fp32, tag="t1")
                nc.vector.tensor_reduce(t1[:], Dp[:, 768:768 + P64],
                                        axis=mybir.AxisListType.XYZW,
                                        op=mybir.AluOpType.add)
                t1s = wpool.tile([BK, 1], fp32, tag="t1s")
                nc.vector.tensor_scalar_mul(t1s[:], t1[:], 1.0 / (C * P64))

                # ---- combine + store ----------------------------------------
                cost_sb = wpool.tile([BK, L], fp32, tag="cost")
                nc.scalar.activation(cost_sb[:, 0:486], Dp[:, 0:486],
                                     mybir.ActivationFunctionType.Identity,
                                     bias=t1s[:], scale=1.0 / (C * P64))
                nc.scalar.activation(cost_sb[:, 486:729], Dp[:, 512:755],
                                     mybir.ActivationFunctionType.Identity,
                                     bias=t1s[:], scale=1.0 / (C * P64))
                nc.sync.dma_start(out_d[b * BK:(b + 1) * BK, :], cost_sb[:])

    nc.compile()
    return nc


def _disp():
    a = np.arange(-DISP_RADIUS, DISP_RADIUS + 1, DISP_STEP)
    g = np.stack(np.meshgrid(a, a, a, indexing="ij"), axis=-1)
    return g.reshape(-1, 3).astype(np.float32)


def kernel(fix_kps, feat_fix, feat_mov):
    fix_kps = np.asarray(fix_kps)
    feat_fix = np.asarray(feat_fix, dtype=np.float32)
    feat_mov = np.asarray(feat_mov, dtype=np.float32)

    with _build_lock:
        if "nc" not in _cache:
            _cache["nc"] = _build_nc()
    nc = _cache["nc"]

    vmov = _pack_mov(feat_mov)
    vfix = _pack_fix(feat_fix)
    ones_bd, sel_y, sel_x, pfmask = _selectors()

    in_maps = []
    for c in range(N_CORES):
        kps = fix_kps[c * KP_PER_CORE:(c + 1) * KP_PER_CORE]
        pad = np.repeat(kps[-1:], NB * BK - KP_PER_CORE, axis=0)
        kpad = np.concatenate([kps, pad], axis=0)
        mz, mr, fz, fr = _offsets_for(kpad)
        row0 = np.stack([mz, mr], axis=1).reshape(-1)
        row1 = np.stack([fz, fr], axis=1).reshape(-1)
        offs = np.stack([row0, row1]).astype(np.int32)
        in_maps.append({
            "vmov": vmov, "vfix": vfix, "offs": offs,
            "ones_bd": ones_bd, "sel_y": sel_y, "sel_x": sel_x,
            "pfmask": pfmask,
        })

    res = run_bass_kernel_spmd(nc, in_maps, list(range(N_CORES)))
    cost = np.concatenate(
        [res.results[c]["cost"][:KP_PER_CORE] for c in range(N_CORES)], axis=0)
    return cost.reshape(N_KPS, 1, LW, LW, LW), _disp()


# revision 18
# speedup vs baseline: 2.6200x; 1.1150x over previous
"""Trainium2 Bass kernel for patch-SSD keypoint matching.

Reference computation (see problem): for each of 512 keypoints, compare a
4x4x4 (stride-2) patch of feat_fix around the keypoint against the same
patch of feat_mov displaced by each of 9x9x9 (stride-2) displacements.
Output cost[N, 1, 9, 9, 9] = mean squared difference over (C=12, P=64),
plus the constant displacement grid [729, 3].

Strategy (8 NeuronCores, data-parallel over keypoints, 64 kps/core):
  * Union of (patch offset + displacement) positions per keypoint is a
    12^3 stride-2 grid -> gather only C*12^3 values per keypoint.
  * cost = (sum pf^2 + box(sum_c pm^2) - 2*corr(pf, pm)) / 768.  The whole
    (term2 - 2*corr) is computed by one block-diagonal matmul pipeline:
    z-handling fused into 4 PSUM-accumulated shifted-rhs matmuls, y/x folds
    by constant 0/1 selector matmuls.
  * Gather: per-keypoint dynamic-offset DMA (base address from registers
    loaded from an SBUF offsets tensor -> SPMD-safe) out of a host-packed
    volume: y/x parity-packed + overlapping x-strips so that each (c, z)
    needs exactly one contiguous 320-element span.
"""

import sys
import threading
from contextlib import ExitStack

if "/opt/trn_rl_repo" not in sys.path:
    sys.path.insert(0, "/opt/trn_rl_repo")

import ml_dtypes
import numpy as np

import concourse.bass as bass
import concourse.bacc as bacc
import concourse.tile as tile
from concourse import mybir
from concourse.bass_utils import run_bass_kernel_spmd

# problem constants
C, D, H, W = 12, 128, 160, 160
N_KPS = 512
DISP_RADIUS, DISP_STEP = 8, 2
PATCH_RADIUS, PATCH_STEP = 3, 2
LW = 9          # displacements per dim
NP_ = 4         # patch taps per dim
G12 = 12        # combined grid points per dim (LW + NP_ - 1)
L = LW ** 3     # 729
P64 = NP_ ** 3  # 64

N_CORES = 8
KP_PER_CORE = N_KPS // N_CORES  # 64
BK = 6                           # keypoints per compute batch
NB = (KP_PER_CORE + BK - 1) // BK  # 11 batches (66 slots, last 2 padded)

# packed-volume geometry
HP = H // 2            # 80 (parity-packed y')
WP = W // 2            # 80 (parity-packed x')
NS = 4                 # x-strips
SW = 28                # strip width
SSTRIDE = 16           # strip stride
SPAN = (G12 - 1) * SW + G12          # 320 contiguous elems per (c, z)
MOVF = 4 * NS * HP * SW              # 35840 flat per (c, z): (par, s, y', x'')
FIXF = 4 * HP * WP                   # 25600 flat per (c, z): (par, y', x')
FSPAN = (NP_ - 1) * WP + NP_         # 244 contiguous per (c, z) for fix

_build_lock = threading.Lock()
_cache = {}


def _pack_mov(feat_mov):
    """[1,C,D,H,W] f32 -> bf16 [C, D, MOVF] with (par, strip, y', x'') packing."""
    fm = np.ascontiguousarray(feat_mov[0]).astype(ml_dtypes.bfloat16)
    t = fm.reshape(C, D, HP, 2, WP, 2)
    t = np.transpose(t, (0, 1, 3, 5, 2, 4))             # [C, D, 2, 2, HP, WP]
    t = t.reshape(C, D, 4, HP, WP)
    v = np.zeros((C, D, 4, NS, HP, SW), dtype=ml_dtypes.bfloat16)
    for s in range(NS):
        w = min(SW, WP - SSTRIDE * s)
        v[:, :, :, s, :, :w] = t[:, :, :, :, SSTRIDE * s:SSTRIDE * s + w]
    return np.ascontiguousarray(v.reshape(C, D, MOVF))


def _pack_fix(feat_fix):
    """[1,C,D,H,W] f32 -> bf16 [C, D, FIXF] with (par, y', x') packing."""
    ff = np.ascontiguousarray(feat_fix[0]).astype(ml_dtypes.bfloat16)
    t = ff.reshape(C, D, HP, 2, WP, 2)
    t = np.transpose(t, (0, 1, 3, 5, 2, 4)).reshape(C, D, 4, HP, WP)
    return np.ascontiguousarray(t.reshape(C, D, FIXF))


def _offsets_for(kps):
    """kps [n,3] int -> (mov_z0, mov_rem, fix_z0, fix_rem) int32 arrays."""
    kz = kps[:, 0].astype(np.int64)
    ky = kps[:, 1].astype(np.int64)
    kx = kps[:, 2].astype(np.int64)
    z0 = kz - (DISP_RADIUS + PATCH_RADIUS)
    y0 = ky - (DISP_RADIUS + PATCH_RADIUS)
    x0 = kx - (DISP_RADIUS + PATCH_RADIUS)
    par = (y0 & 1) * 2 + (x0 & 1)
    yq = y0 >> 1
    xq = x0 >> 1
    s = xq >> 4
    xr = xq & 15
    mov_z0 = z0
    mov_rem = par * (NS * HP * SW) + s * (HP * SW) + yq * SW + xr
    fix_z0 = kz - PATCH_RADIUS
    fix_rem = par * (HP * WP) + (yq + 4) * WP + (xq + 4)
    return (mov_z0.astype(np.int32), mov_rem.astype(np.int32),
            fix_z0.astype(np.int32), fix_rem.astype(np.int32))


def _offs_rows(mz, mr, fz, fr):
    """Pack offsets as [2, 12*NB]: per batch, ring-r gets
    [mz,mr for its 3 mov kps] + [fz,fr for its 3 fix kps]."""
    rows = np.zeros((2, 12 * NB), dtype=np.int32)
    for b in range(NB):
        for ri in range(2):
            movk = (0, 2, 4) if ri == 0 else (1, 3, 5)
            fixk = (1, 3, 5) if ri == 0 else (0, 2, 4)
            vals = []
            for k in movk:
                vals += [mz[b * BK + k], mr[b * BK + k]]
            for k in fixk:
                vals += [fz[b * BK + k], fr[b * BK + k]]
            rows[ri, 12 * b:12 * b + 12] = vals
    return rows


def _selectors():
    """Constant block-diagonal / selector matrices (host-built)."""
    ones_bd = np.zeros((BK * C, BK), dtype=np.float32)
    for k in range(BK):
        ones_bd[C * k:C * k + C, k] = 1.0
    # y-fold input rows: [0:96] = cost3 rows, [96:102] = box rows
    sel_y = np.zeros((BK * 16 + BK, 4, BK * 4 + BK), dtype=np.float32)
    for k in range(BK):
        for pyv in range(4):
            for px in range(4):
                sel_y[16 * k + 4 * pyv + px, pyv, 4 * k + px] = 1.0
        for pyv in range(4):
            sel_y[BK * 16 + k, pyv, BK * 4 + k] = 1.0
    sel_x = np.zeros((BK * 4 + BK, 4, BK), dtype=np.float32)
    for k in range(BK):
        for px in range(4):
            sel_x[4 * k + px, px, k] = 1.0
        for px in range(4):
            sel_x[BK * 4 + k, px, k] = 1.0
    # block-diagonal -2 mask for the lhsT build: row (kp,c), col (kp',py,px)
    pfmask = np.zeros((BK * C, BK * 16), dtype=np.float32)
    for k in range(BK):
        pfmask[C * k:C * k + C, 16 * k:16 * k + 16] = -2.0
    b16 = ml_dtypes.bfloat16
    return (ones_bd.astype(b16),
            np.ascontiguousarray(sel_y.reshape(BK * 16 + BK, 4 * (BK * 4 + BK))).astype(b16),
            np.ascontiguousarray(sel_x.reshape(BK * 4 + BK, 4 * BK)).astype(b16),
            pfmask.astype(b16))


def _build_nc():
    """Build + compile the SPMD kernel once (shapes are static)."""
    fp32 = mybir.dt.float32
    bf16 = mybir.dt.bfloat16
    i32 = mybir.dt.int32
    KC = BK * C                      # 72 partitions (kp, c)
    M1 = BK * 16                     # 96 z-stage output partitions
    MY = BK * 4 + BK                 # 30 y-fold output partitions
    NYR = BK * 16 + BK               # 102 y-fold input partitions (cost3@[0:96], box@[96:102])

    nc = bacc.Bacc("TRN2", target_bir_lowering=False, debug=False)
    vm = nc.dram_tensor("vmov", [C, D, MOVF], bf16, kind="ExternalInput")
    vf = nc.dram_tensor("vfix", [C, D, FIXF], bf16, kind="ExternalInput")
    offs = nc.dram_tensor("offs", [2, 12 * NB], i32, kind="ExternalInput")
    ones_d = nc.dram_tensor("ones_bd", [KC, BK], bf16, kind="ExternalInput")
    sely_d = nc.dram_tensor("sel_y", [NYR, 4 * MY], bf16, kind="ExternalInput")
    selx_d = nc.dram_tensor("sel_x", [MY, 4 * BK], bf16, kind="ExternalInput")
    pfmask_d = nc.dram_tensor("pfmask", [KC, M1], bf16, kind="ExternalInput")
    out_d = nc.dram_tensor("cost", [NB * BK, L], fp32, kind="ExternalOutput")

    with tile.TileContext(nc) as tc:
        with (
            tc.tile_pool(name="const", bufs=1) as cpool,
            tc.tile_pool(name="gather", bufs=3) as gpool,
            tc.tile_pool(name="work", bufs=3) as wpool,
            tc.tile_pool(name="lhs", bufs=2) as lpool,
            tc.tile_pool(name="psB", bufs=1, space="PSUM") as psB,
            tc.tile_pool(name="psC", bufs=1, space="PSUM") as psC,
            tc.tile_pool(name="psD", bufs=1, space="PSUM") as psD,
        ):
            offs_sb = cpool.tile([2, 12 * NB], i32)
            nc.sync.dma_start(offs_sb[:], offs[:])
            ones_sb = cpool.tile([KC, BK], bf16)
            nc.sync.dma_start(ones_sb[:], ones_d[:])
            sely_sb = cpool.tile([NYR, 4 * MY], bf16)
            nc.sync.dma_start(sely_sb[:], sely_d[:])
            selx_sb = cpool.tile([MY, 4 * BK], bf16)
            nc.sync.dma_start(selx_sb[:], selx_d[:])
            pfmask_sb = cpool.tile([KC, M1], bf16)
            nc.sync.dma_start(pfmask_sb[:], pfmask_d[:])

            for b in range(NB):
                # ---- gathers -------------------------------------------------
                g6 = gpool.tile([KC, G12, SPAN], bf16, tag="g6")
                pf6 = gpool.tile([KC, NP_, FSPAN], bf16, tag="pf6")
                # offs row r (r=0 sync / r=1 scalar), 12 values per batch:
                # [mz,mr for 3 mov kps] + [fz,fr for 3 fix kps]
                for ri, eng in ((0, nc.sync), (1, nc.scalar)):
                    with ExitStack() as rstk:
                        regs = [rstk.enter_context(
                            eng.register(f"r{b}_{ri}_{i}")) for i in range(12)]
                        eng.reg_load(regs, offs_sb[ri:ri + 1, 12 * b:12 * b + 12])
                        movk = (0, 2, 4) if ri == 0 else (1, 3, 5)
                        fixk = (1, 3, 5) if ri == 0 else (0, 2, 4)
                        for i, k in enumerate(movk):
                            zv = eng.snap(regs[2 * i], min_val=0,
                                          max_val=D - 2 * G12 + 1)
                            rv = eng.snap(regs[2 * i + 1], min_val=0,
                                          max_val=MOVF - SPAN)
                            eng.dma_start(
                                g6[C * k:C * k + C, :, :],
                                vm[:, bass.ds(zv, G12, 2), bass.ds(rv, SPAN)])
                        for i, k in enumerate(fixk):
                            zv = eng.snap(regs[6 + 2 * i], min_val=0,
                                          max_val=D - 2 * NP_ + 1)
                            rv = eng.snap(regs[7 + 2 * i], min_val=0,
                                          max_val=FIXF - FSPAN)
                            eng.dma_start(
                                pf6[C * k:C * k + C, :, :],
                                vf[:, bass.ds(zv, NP_, 2), bass.ds(rv, FSPAN)])

                # ---- lhsT build: -2 * PF block-diag via broadcast * mask
                lhsT = lpool.tile([KC, 4, M1], bf16, tag="lhsT")
                for pz in range(4):
                    src = bass.AP(pf6.tensor, pz * FSPAN,
                                  [[pf6[:].ap[0][0], KC],
                                   [0, BK], [WP, NP_], [1, NP_]])
                    nc.vector.tensor_tensor(
                        out=lhsT[:, pz, :], in0=src, in1=pfmask_sb[:],
                        op=mybir.AluOpType.mult)

                # ---- squared-moving path ------------------------------------
                g2 = wpool.tile([KC, G12 * G12 * G12], bf16, tag="g2")
                src = bass.AP(g6.tensor, g6[0, 0, 0].offset,
                              [[g6[:].ap[0][0], KC],
                               [SPAN, G12], [SW, G12], [1, G12]])
                nc.scalar.activation(g2[:], src, mybir.ActivationFunctionType.Square)
                zb1 = wpool.tile([KC, LW * G12 * G12], bf16, tag="zb1")
                zb2 = wpool.tile([KC, LW * G12 * G12], bf16, tag="zb2")
                ZR = G12 * G12  # 144
                nc.vector.tensor_add(
                    zb1[:], g2[:, 0:LW * ZR], g2[:, ZR:(LW + 1) * ZR])
                nc.vector.tensor_add(
                    zb2[:], g2[:, 2 * ZR:(LW + 2) * ZR], g2[:, 3 * ZR:(LW + 3) * ZR])
                nc.vector.tensor_add(zb1[:], zb1[:], zb2[:])

                # ---- z-stage + box c-reduction into PSUM B -------------------
                Bp = psB.tile([M1, 3 * 512], fp32, tag="B")
                for pz in range(4):
                    for ck in range(3):
                        rhs = bass.AP(g6.tensor, g6[0, pz + 3 * ck, 0].offset,
                                      [[g6[:].ap[0][0], KC],
                                       [SPAN, 3], [SW, G12], [1, G12]])
                        nc.tensor.matmul(
                            Bp[0:M1, 512 * ck:512 * ck + 432],
                            lhsT[:, pz, :], rhs,
                            start=(pz == 0), stop=(pz == 3))
                boxP = psC.tile([MY, 3 * 512], fp32, tag="C")
                for ck in range(3):
                    nc.tensor.matmul(
                        boxP[0:BK, 512 * ck:512 * ck + 432],
                        ones_sb[:], zb1[:, 432 * ck:432 * ck + 432],
                        start=True, stop=True)

                # ---- B -> SBUF (cost3 rows via ACT, box rows via DVE quadrant move)
                bsb = wpool.tile([NYR, 1296], bf16, tag="bsb")
                nc.scalar.copy(
                    bsb[0:M1, :],
                    bass.AP(Bp.tensor, Bp[0, 0].offset,
                            [[Bp[:].ap[0][0], M1], [512, 3], [1, 432]]))
                nc.vector.tensor_copy(
                    bsb[M1:M1 + BK, :],
                    bass.AP(boxP.tensor, boxP[0, 0].offset,
                            [[boxP[:].ap[0][0], BK], [512, 3], [1, 432]]))

                # ---- y-fold --------------------------------------------------
                Cp = psC.tile([MY, 2 * 512], fp32, tag="C")
                for pyv in range(4):
                    for ck in range(2):
                        rhs = bass.AP(bsb.tensor, bsb[0, pyv * G12 + 6 * ck].offset,
                                      [[bsb[:].ap[0][0], NYR],
                                       [ZR, LW], [G12, LW], [1, 6]])
                        nc.tensor.matmul(
                            Cp[:, 512 * ck:512 * ck + 486],
                            sely_sb[:, MY * pyv:MY * pyv + MY], rhs,
                            start=(pyv == 0), stop=(pyv == 3))
                csb = wpool.tile([MY, LW * LW * G12], bf16, tag="csb")
                for ck in range(2):
                    nc.scalar.copy(
                        bass.AP(csb.tensor, csb[0, 6 * ck].offset,
                                [[csb[:].ap[0][0], MY], [G12, LW * LW], [1, 6]]),
                        Cp[:, 512 * ck:512 * ck + 486])

                # ---- x-fold + term1 -----------------------------------------
                Dp = psD.tile([BK, 1024], fp32, tag="D")
                xcw = (486, 243)
                xcl = (6, 3)
                YR = LW * G12  # 108
                for px in range(4):
                    lz0 = 0
                    for ck in range(2):
                        rhs = bass.AP(csb.tensor, csb[0, lz0 * YR + px].offset,
                                      [[csb[:].ap[0][0], MY],
                                       [YR, xcl[ck]], [G12, LW], [1, LW]])
                        nc.tensor.matmul(
                            Dp[:, 512 * ck:512 * ck + xcw[ck]],
                            selx_sb[:, BK * px:BK * px + BK], rhs,
                            start=(px == 0), stop=(px == 3))
                        lz0 += xcl[ck]

                pfsq = wpool.tile([KC, P64], bf16, tag="pfsq")
                src = bass.AP(pf6.tensor, pf6[0, 0, 0].offset,
                              [[pf6[:].ap[0][0], KC],
                               [FSPAN, NP_], [WP, NP_], [1, NP_]])
                nc.scalar.activation(pfsq[:], src, mybir.ActivationFunctionType.Square)
                nc.tensor.matmul(Dp[:, 768:768 + P64], ones_sb[:], pfsq[:],
                                 start=True, stop=True)
                t1 = wpool.tile([BK, 1], fp32, tag="t1")
                nc.vector.tensor_reduce(t1[:], Dp[:, 768:768 + P64],
                                        axis=mybir.AxisListType.XYZW,
                                        op=mybir.AluOpType.add)
                t1s = wpool.tile([BK, 1], fp32, tag="t1s")
                nc.vector.tensor_scalar_mul(t1s[:], t1[:], 1.0 / (C * P64))

                # ---- combine + store ----------------------------------------
                cost_sb = wpool.tile([BK, L], fp32, tag="cost")
                nc.scalar.activation(cost_sb[:, 0:486], Dp[:, 0:486],
                                     mybir.ActivationFunctionType.Identity,
                                     bias=t1s[:], scale=1.0 / (C * P64))
                nc.scalar.activation(cost_sb[:, 486:729], Dp[:, 512:755],
                                     mybir.ActivationFunctionType.Identity,
                                     bias=t1s[:], scale=1.0 / (C * P64))
                nc.sync.dma_start(out_d[b * BK:(b + 1) * BK, :], cost_sb[:])

    nc.compile()
    return nc


def _disp():
    a = np.arange(-DISP_RADIUS, DISP_RADIUS + 1, DISP_STEP)
    g = np.stack(np.meshgrid(a, a, a, indexing="ij"), axis=-1)
    return g.reshape(-1, 3).astype(np.float32)


def kernel(fix_kps, feat_fix, feat_mov):
    fix_kps = np.asarray(fix_kps)
    feat_fix = np.asarray(feat_fix, dtype=np.float32)
    feat_mov = np.asarray(feat_mov, dtype=np.float32)

    with _build_lock:
        if "nc" not in _cache:
            _cache["nc"] = _build_nc()
    nc = _cache["nc"]

    vmov = _pack_mov(feat_mov)
    vfix = _pack_fix(feat_fix)
    ones_bd, sel_y, sel_x, pfmask = _selectors()

    in_maps = []
    for c in range(N_CORES):
        kps = fix_kps[c * KP_PER_CORE:(c + 1) * KP_PER_CORE]
        pad = np.repeat(kps[-1:], NB * BK - KP_PER_CORE, axis=0)
        kpad = np.concatenate([kps, pad], axis=0)
        mz, mr, fz, fr = _offsets_for(kpad)
        offs = _offs_rows(mz, mr, fz, fr)
        in_maps.append({
            "vmov": vmov, "vfix": vfix, "offs": offs,
            "ones_bd": ones_bd, "sel_y": sel_y, "sel_x": sel_x,
            "pfmask": pfmask,
        })

    res = run_bass_kernel_spmd(nc, in_maps, list(range(N_CORES)))
    cost = np.concatenate(
        [res.results[c]["cost"][:KP_PER_CORE] for c in range(N_CORES)], axis=0)
    return cost.reshape(N_KPS, 1, LW, LW, LW), _disp()
